# revision 1
# baseline (speedup 1.0000x reference)
"""GCN (3-layer, skip-concat) on 8 Trainium2 NeuronCores.

Strategy (hardcoded for N=10000, E=320000, dims 512/512/256):
  - Row-partition nodes across 8 cores (1280 padded rows each, N padded
    to 10240).
  - The adjacency shard is densified on the host into A_k^T [10240, 1280]
    (bf16): SpMM becomes a dense matmul on TensorE, which beats any
    gather/scatter scheme here (random graph, 0.32% density -> every
    128-wide tile is populated).
  - Activations live feature-major (X^T) in SBUF. Per layer:
      S_k   = X^T.T @ W          (node-major [1280, d_out], PE)
      S     = AllGather(S_k)     (bf16, HBM collective)
      Y^T   = S_tiles^T @ A_k^T  (feature-major, PE; S tiles stationary)
      X' ^T = [relu(Y^T + b); (Y^T + b)]   (partition-axis concat, free)
  - All matmuls bf16 with fp32 PSUM accumulation (rel err ~1e-3).
"""

import os
import numpy as np
from ml_dtypes import bfloat16

N = 10000
NPAD = 10240
NCORES = 8
R = NPAD // NCORES  # 1280 rows per core
P = 128
CT = NPAD // P  # 80 contraction tiles for the SpMM

D0_IN, D0_OUT = 512, 512
D1_IN, D1_OUT = 1024, 512
D2_IN, D2_OUT = 1024, 256

# n-chunks of the 1280-wide free dim (PSUM bank = 512 fp32)
N_CHUNKS = [(0, 512), (512, 512), (1024, 256)]

_CACHE = {}
LAST_RESULT = None  # BassKernelResults of the most recent run (for test.py)


def _build_bass():
    import concourse.bass as bass
    import concourse.bacc as bacc
    import concourse.mybir as mybir
    import concourse.tile as tile

    dt = mybir.dt
    bf16 = dt.bfloat16
    f32 = dt.float32
    ts = bass.ts

    nc = bacc.Bacc(
        "TRN2",
        target_bir_lowering=False,
        debug=False,
        enable_asserts=False,
        num_devices=NCORES,
    )

    xT_d = nc.dram_tensor("xT", [D0_IN, R], bf16, kind="ExternalInput")
    adjT_d = nc.dram_tensor("adjT", [NPAD, R], bf16, kind="ExternalInput")
    W_d = [
        nc.dram_tensor("W0", [D0_IN, D0_OUT], bf16, kind="ExternalInput"),
        nc.dram_tensor("W1", [D1_IN, D1_OUT], bf16, kind="ExternalInput"),
        nc.dram_tensor("W2", [D2_IN, D2_OUT], bf16, kind="ExternalInput"),
    ]
    b_d = [
        nc.dram_tensor("b0", [D0_OUT, 1], f32, kind="ExternalInput"),
        nc.dram_tensor("b1", [D1_OUT, 1], f32, kind="ExternalInput"),
        nc.dram_tensor("b2", [D2_OUT, 1], f32, kind="ExternalInput"),
    ]
    outT_d = nc.dram_tensor("outT", [D2_OUT, R], f32, kind="ExternalOutput")

    layers = [
        (D0_IN, D0_OUT),
        (D1_IN, D1_OUT),
        (D2_IN, D2_OUT),
    ]

    with tile.TileContext(nc) as tc:
        with (
            tc.tile_pool(name="persist", bufs=1) as persist,
            tc.tile_pool(name="work", bufs=3) as work,
            tc.tile_pool(name="psum", bufs=1, space="PSUM") as psum_pool,
            tc.tile_pool(name="dram", bufs=1, space="DRAM") as dram_pool,
        ):
            # ---- resident weights / biases ----
            w_sb = []
            for L, (d_in, d_out) in enumerate(layers):
                tiles = []
                for c in range(d_in // P):
                    wt = persist.tile(
                        [P, d_out], bf16, name=f"w{L}_{c}", tag=f"w{L}_{c}"
                    )
                    nc.sync.dma_start(wt[:], W_d[L][ts(c, P), :])
                    tiles.append(wt)
                w_sb.append(tiles)

            b_sb = []
            for L, (d_in, d_out) in enumerate(layers):
                tiles = []
                for pi in range(d_out // P):
                    bt = persist.tile([P, 1], f32, name=f"b{L}_{pi}", tag=f"b{L}_{pi}")
                    nc.sync.dma_start(bt[:], b_d[L][ts(pi, P), :])
                    tiles.append(bt)
                b_sb.append(tiles)

            # ---- initial activations X^T (feature-major, bf16) ----
            xt = []
            for c in range(D0_IN // P):
                t = persist.tile([P, R], bf16, name=f"xt0_{c}", tag=f"xt0_{c}")
                nc.sync.dma_start(t[:], xT_d[ts(c, P), :])
                xt.append(t)

            for L, (d_in, d_out) in enumerate(layers):
                n_ct = d_in // P
                n_po = d_out // P

                # ---- dense: S_k[m*128:(m+1)*128, :] = X^T.T @ W ----
                s_bounce = dram_pool.tile(
                    [R, d_out], bf16, name=f"s_bounce{L}", tag=f"s_bounce{L}"
                )
                for m in range(R // P):
                    dps = psum_pool.tile(
                        [P, d_out], f32, name=f"dense_ps_{L}_{m}", tag="dense_ps",
                        bufs=2,
                    )
                    for c in range(n_ct):
                        nc.tensor.matmul(
                            dps[:],
                            lhsT=xt[c][:, ts(m, P)],
                            rhs=w_sb[L][c][:],
                            start=(c == 0),
                            stop=(c == n_ct - 1),
                        )
                    s_sb = work.tile(
                        [P, d_out], bf16, name=f"s_sb_{L}_{m}", tag="s_sb", bufs=3
                    )
                    nc.vector.tensor_copy(s_sb[:], dps[:])
                    nc.sync.dma_start(s_bounce[ts(m, P), :], s_sb[:])

                # ---- all-gather S across the 8 cores ----
                s_all = dram_pool.tile(
                    [NPAD, d_out],
                    bf16,
                    name=f"s_all{L}",
                    tag=f"s_all{L}",
                    addr_space="Shared",
                )
                nc.gpsimd.collective_compute(
                    "AllGather",
                    mybir.AluOpType.bypass,
                    replica_groups=[list(range(NCORES))],
                    ins=[s_bounce.opt()],
                    outs=[s_all.opt()],
                )

                # ---- load gathered S into SBUF (stationary operand) ----
                s_tiles = []
                for ct in range(CT):
                    st = persist.tile([P, 512], bf16, name=f"s_{ct}", tag=f"s_{ct}")
                    nc.sync.dma_start(st[:, :d_out], s_all[ts(ct, P), :])
                    s_tiles.append(st)

                # ---- next-layer activations (feature-major) ----
                if L < 2:
                    xt_next = []
                    for c in range(2 * n_po):
                        t = persist.tile(
                            [P, R], bf16, name=f"xt{L + 1}_{c}", tag=f"xt{L + 1}_{c}"
                        )
                        xt_next.append(t)

                # ---- SpMM: Y^T = S^T @ A_k^T, n-chunked over the 1280 free ----
                for nci, (n0, nw) in enumerate(N_CHUNKS):
                    sp_ps = [
                        psum_pool.tile(
                            [P, nw], f32, name=f"sp_{L}_{nci}_{p}", tag=f"sp{p}"
                        )
                        for p in range(n_po)
                    ]
                    for ct in range(CT):
                        at = work.tile(
                            [P, 512], bf16, name=f"at_{L}_{nci}_{ct}", tag="at", bufs=6
                        )
                        nc.sync.dma_start(
                            at[:, :nw], adjT_d[ts(ct, P), n0 : n0 + nw]
                        )
                        for p in range(n_po):
                            nc.tensor.matmul(
                                sp_ps[p][:],
                                lhsT=s_tiles[ct][:, ts(p, P)],
                                rhs=at[:, :nw],
                                start=(ct == 0),
                                stop=(ct == CT - 1),
                            )
                    # epilogue: bias, relu, concat (or final output)
                    for p in range(n_po):
                        if L < 2:
                            nc.scalar.activation(
                                xt_next[p][:, n0 : n0 + nw],
                                sp_ps[p][:],
                                mybir.ActivationFunctionType.Relu,
                                bias=b_sb[L][p][:],
                            )
                            nc.vector.tensor_scalar_add(
                                xt_next[n_po + p][:, n0 : n0 + nw],
                                sp_ps[p][:],
                                b_sb[L][p][:],
                            )
                        else:
                            ot = work.tile(
                                [P, nw], f32, name=f"ot_{nci}_{p}", tag="ot", bufs=3
                            )
                            nc.vector.tensor_scalar_add(
                                ot[:], sp_ps[p][:], b_sb[L][p][:]
                            )
                            nc.sync.dma_start(
                                outT_d[ts(p, P), n0 : n0 + nw], ot[:]
                            )
                if L < 2:
                    xt = xt_next

    nc.compile()
    return nc


def _get_nc():
    if "nc" not in _CACHE:
        _CACHE["nc"] = _build_bass()
    return _CACHE["nc"]


def _preprocess(x, edge_row, edge_col, edge_val, W0, W1, W2, b0, b1, b2):
    x = np.asarray(x, np.float32)
    edge_row = np.asarray(edge_row, np.int64)
    edge_col = np.asarray(edge_col, np.int64)
    edge_val = np.asarray(edge_val, np.float32)

    # dense per-core adjacency blocks, transposed: adjT[k][c, r_local]
    adjT = np.zeros((NCORES, NPAD, R), np.float32)
    core = edge_row // R
    r_local = edge_row % R
    np.add.at(adjT, (core, edge_col, r_local), edge_val)
    adjT = adjT.astype(bfloat16)

    x_pad = np.zeros((NPAD, x.shape[1]), np.float32)
    x_pad[:N] = x

    in_maps = []
    for k in range(NCORES):
        xT_k = np.ascontiguousarray(x_pad[k * R : (k + 1) * R].T).astype(bfloat16)
        in_maps.append(
            {
                "xT": xT_k,
                "adjT": np.ascontiguousarray(adjT[k]),
                "W0": np.asarray(W0, np.float32).astype(bfloat16),
                "W1": np.asarray(W1, np.float32).astype(bfloat16),
                "W2": np.asarray(W2, np.float32).astype(bfloat16),
                "b0": np.asarray(b0, np.float32).reshape(-1, 1),
                "b1": np.asarray(b1, np.float32).reshape(-1, 1),
                "b2": np.asarray(b2, np.float32).reshape(-1, 1),
            }
        )
    return in_maps


def kernel(x, edge_row, edge_col, edge_val, W0, W1, W2, b0, b1, b2):
    global LAST_RESULT
    from concourse.bass_utils import run_bass_kernel_spmd

    nc = _get_nc()
    in_maps = _preprocess(
        x, edge_row, edge_col, edge_val, W0, W1, W2, b0, b1, b2
    )
    res = run_bass_kernel_spmd(
        nc,
        in_maps,
        core_ids=list(range(NCORES)),
        trace=bool(int(os.environ.get("GCN_TRACE", "0"))),
    )
    LAST_RESULT = res

    outT = np.concatenate(
        [np.asarray(res.results[k]["outT"]) for k in range(NCORES)], axis=1
    )  # [256, 10240]
    return np.ascontiguousarray(outT.T[:N]).astype(np.float32)


# revision 2
# speedup vs baseline: 1.1206x; 1.1206x over previous
"""GCN (3-layer, skip-concat) on 8 Trainium2 NeuronCores.

Strategy (hardcoded for N=10000, E=320000, dims 512/512/256):
  - Row-partition nodes across 8 cores (1280 padded rows each, N padded
    to 10240).
  - The adjacency shard is densified on the host into A_k^T [10240, 1280]
    (bf16): SpMM becomes a dense matmul on TensorE, which beats any
    gather/scatter scheme here (random graph, 0.32% density -> every
    128-wide tile is populated).
  - Activations live feature-major (X^T) in SBUF. Per layer:
      S_k   = X^T.T @ W          (node-major [1280, d_out], PE)
      S     = AllGather(S_k)     (bf16, HBM collective, 5 chunks
                                  pipelined against compute)
      Y^T   = S_tiles^T @ A_k^T  (feature-major, PE; S tiles stationary)
      X' ^T = [relu(Y^T + b); (Y^T + b)]   (partition-axis concat, free)
  - The AllGather is chunked (5 x 256 rows per rank); the adjacency's
    contraction rows are permuted on the host so chunk j of the gather
    covers contraction tiles [16j, 16j+16). Chunk j's input only needs
    dense m-tiles 2j,2j+1, so for layers 1/2 the gathers complete while
    the previous SpMM still runs.
  - adjacency blocks are pre-tiled contiguously on the host: one DMA per
    [128 x nw] tile.
  - All matmuls bf16 with fp32 PSUM accumulation (rel err ~2e-3).
"""

import os
import numpy as np
from ml_dtypes import bfloat16

N = 10000
NPAD = 10240
NCORES = 8
R = NPAD // NCORES  # 1280 rows per core
P = 128
CT = NPAD // P  # 80 contraction tiles for the SpMM

NAG = 5  # all-gather chunks per layer
AGR = R // NAG  # 256 rows per rank per chunk
AGT = NPAD // NAG // P  # 16 contraction tiles per gathered chunk

D0_IN, D0_OUT = 512, 512
D1_IN, D1_OUT = 1024, 512
D2_IN, D2_OUT = 1024, 256

# n-chunks of the 1280-wide free dim (PSUM bank = 512 fp32)
N_CHUNKS = [(0, 512), (512, 512), (1024, 256)]

_CACHE = {}
LAST_RESULT = None  # BassKernelResults of the most recent run (for test.py)


def _build_bass():
    import concourse.bass as bass
    import concourse.bacc as bacc
    import concourse.mybir as mybir
    import concourse.tile as tile

    dt = mybir.dt
    bf16 = dt.bfloat16
    f32 = dt.float32
    ts = bass.ts

    nc = bacc.Bacc(
        "TRN2",
        target_bir_lowering=False,
        debug=False,
        enable_asserts=False,
        num_devices=NCORES,
    )

    xT_d = nc.dram_tensor("xT", [D0_IN, R], bf16, kind="ExternalInput")
    # pre-tiled adjacency: chunks 0,1 (512 wide) and chunk 2 (256 wide)
    adjT01_d = nc.dram_tensor(
        "adjT01", [2, NAG, AGT, P, 512], bf16, kind="ExternalInput"
    )
    adjT2_d = nc.dram_tensor("adjT2", [NAG, AGT, P, 256], bf16, kind="ExternalInput")
    W_d = [
        nc.dram_tensor("W0", [D0_IN, D0_OUT], bf16, kind="ExternalInput"),
        nc.dram_tensor("W1", [D1_IN, D1_OUT], bf16, kind="ExternalInput"),
        nc.dram_tensor("W2", [D2_IN, D2_OUT], bf16, kind="ExternalInput"),
    ]
    b_d = [
        nc.dram_tensor("b0", [D0_OUT, 1], f32, kind="ExternalInput"),
        nc.dram_tensor("b1", [D1_OUT, 1], f32, kind="ExternalInput"),
        nc.dram_tensor("b2", [D2_OUT, 1], f32, kind="ExternalInput"),
    ]
    outT_d = nc.dram_tensor("outT", [D2_OUT, R], f32, kind="ExternalOutput")

    layers = [
        (D0_IN, D0_OUT),
        (D1_IN, D1_OUT),
        (D2_IN, D2_OUT),
    ]

    with tile.TileContext(nc) as tc:
        with (
            tc.tile_pool(name="persist", bufs=1) as persist,
            tc.tile_pool(name="work", bufs=3) as work,
            tc.tile_pool(name="psum", bufs=1, space="PSUM") as psum_pool,
            tc.tile_pool(name="dram", bufs=1, space="DRAM") as dram_pool,
        ):
            # ---- resident weights / biases ----
            w_sb = []
            for L, (d_in, d_out) in enumerate(layers):
                tiles = []
                for c in range(d_in // P):
                    wt = persist.tile(
                        [P, d_out], bf16, name=f"w{L}_{c}", tag=f"w{L}_{c}"
                    )
                    nc.sync.dma_start(wt[:], W_d[L][ts(c, P), :])
                    tiles.append(wt)
                w_sb.append(tiles)

            b_sb = []
            for L, (d_in, d_out) in enumerate(layers):
                tiles = []
                for pi in range(d_out // P):
                    bt = persist.tile([P, 1], f32, name=f"b{L}_{pi}", tag=f"b{L}_{pi}")
                    nc.sync.dma_start(bt[:], b_d[L][ts(pi, P), :])
                    tiles.append(bt)
                b_sb.append(tiles)

            # ---- initial activations X^T (feature-major, bf16) ----
            xt = []
            for c in range(D0_IN // P):
                t = persist.tile([P, R], bf16, name=f"xt0_{c}", tag=f"xt0_{c}")
                nc.sync.dma_start(t[:], xT_d[ts(c, P), :])
                xt.append(t)

            for L, (d_in, d_out) in enumerate(layers):
                n_ct = d_in // P
                n_po = d_out // P

                # ---- dense + chunked all-gather, pipelined ----
                # chunk j needs dense m-tiles 2j, 2j+1
                s_bounce = dram_pool.tile(
                    [R, d_out], bf16, name=f"s_bounce{L}", tag=f"s_bounce{L}"
                )
                s_all = []
                for j in range(NAG):
                    for m in (2 * j, 2 * j + 1):
                        dps = psum_pool.tile(
                            [P, d_out], f32, name=f"dense_ps_{L}_{m}",
                            tag="dense_ps", bufs=2,
                        )
                        for c in range(n_ct):
                            nc.tensor.matmul(
                                dps[:],
                                lhsT=xt[c][:, ts(m, P)],
                                rhs=w_sb[L][c][:],
                                start=(c == 0),
                                stop=(c == n_ct - 1),
                            )
                        s_sb = work.tile(
                            [P, d_out], bf16, name=f"s_sb_{L}_{m}", tag="s_sb", bufs=4
                        )
                        nc.vector.tensor_copy(s_sb[:], dps[:])
                        nc.sync.dma_start(s_bounce[ts(m, P), :], s_sb[:])
                    s_all_j = dram_pool.tile(
                        [NAG * AGR * 0 + NCORES * AGR, d_out],
                        bf16,
                        name=f"s_all{L}_{j}",
                        tag=f"s_all{L}_{j}",
                        addr_space="Shared",
                    )
                    nc.gpsimd.collective_compute(
                        "AllGather",
                        mybir.AluOpType.bypass,
                        replica_groups=[list(range(NCORES))],
                        ins=[s_bounce[ts(j, AGR), :].opt()],
                        outs=[s_all_j.opt()],
                    )
                    s_all.append(s_all_j)

                # ---- load gathered S into SBUF (stationary operand) ----
                s_tiles = []
                for j in range(NAG):
                    for t in range(AGT):
                        ct = j * AGT + t
                        st = persist.tile([P, 512], bf16, name=f"s_{ct}", tag=f"s_{ct}")
                        nc.sync.dma_start(st[:, :d_out], s_all[j][ts(t, P), :])
                        s_tiles.append(st)

                # ---- next-layer activations (feature-major) ----
                if L < 2:
                    xt_next = []
                    for c in range(2 * n_po):
                        t = persist.tile(
                            [P, R], bf16, name=f"xt{L + 1}_{c}", tag=f"xt{L + 1}_{c}"
                        )
                        xt_next.append(t)

                # ---- SpMM: Y^T = S^T @ A_k^T, n-chunked over the 1280 free ----
                for nci, (n0, nw) in enumerate(N_CHUNKS):
                    sp_ps = [
                        psum_pool.tile(
                            [P, nw], f32, name=f"sp_{L}_{nci}_{p}", tag=f"sp{p}"
                        )
                        for p in range(n_po)
                    ]
                    for j in range(NAG):
                        for t in range(AGT):
                            ct = j * AGT + t
                            at = work.tile(
                                [P, 512], bf16, name=f"at_{L}_{nci}_{ct}", tag="at",
                                bufs=12,
                            )
                            if nci < 2:
                                nc.sync.dma_start(at[:], adjT01_d[nci, j, t])
                            else:
                                nc.sync.dma_start(at[:, :nw], adjT2_d[j, t])
                            for p in range(n_po):
                                nc.tensor.matmul(
                                    sp_ps[p][:],
                                    lhsT=s_tiles[ct][:, ts(p, P)],
                                    rhs=at[:, :nw],
                                    start=(ct == 0),
                                    stop=(ct == CT - 1),
                                )
                    # epilogue: bias, relu, concat (or final output)
                    for p in range(n_po):
                        if L < 2:
                            nc.scalar.activation(
                                xt_next[p][:, n0 : n0 + nw],
                                sp_ps[p][:],
                                mybir.ActivationFunctionType.Relu,
                                bias=b_sb[L][p][:],
                            )
                            nc.vector.tensor_scalar_add(
                                xt_next[n_po + p][:, n0 : n0 + nw],
                                sp_ps[p][:],
                                b_sb[L][p][:],
                            )
                        else:
                            ot = work.tile(
                                [P, nw], f32, name=f"ot_{nci}_{p}", tag="ot", bufs=3
                            )
                            nc.vector.tensor_scalar_add(
                                ot[:], sp_ps[p][:], b_sb[L][p][:]
                            )
                            nc.sync.dma_start(
                                outT_d[ts(p, P), n0 : n0 + nw], ot[:]
                            )
                if L < 2:
                    xt = xt_next

    nc.compile()
    return nc


def _get_nc():
    if "nc" not in _CACHE:
        _CACHE["nc"] = _build_bass()
    return _CACHE["nc"]


def _preprocess(x, edge_row, edge_col, edge_val, W0, W1, W2, b0, b1, b2):
    x = np.asarray(x, np.float32)
    edge_row = np.asarray(edge_row, np.int64)
    edge_col = np.asarray(edge_col, np.int64)
    edge_val = np.asarray(edge_val, np.float32)

    # contraction permutation for the chunked all-gather:
    # new index j*2048 + k*256 + r  <->  old (global node) k*1280 + j*256 + r
    jj, kk, rr = np.meshgrid(
        np.arange(NAG), np.arange(NCORES), np.arange(AGR), indexing="ij"
    )
    new_of_old = np.empty(NPAD, np.int64)
    new_of_old[(kk * R + jj * AGR + rr).ravel()] = (
        jj * (NCORES * AGR) + kk * AGR + rr
    ).ravel()

    # dense per-core adjacency blocks, transposed + permuted:
    # adjT[k][new_of_old[c], r_local] = sum of vals of edges (k*R+r_local, c)
    adjT = np.zeros((NCORES, NPAD, R), np.float32)
    core = edge_row // R
    r_local = edge_row % R
    np.add.at(adjT, (core, new_of_old[edge_col], r_local), edge_val)
    adjT = adjT.astype(bfloat16)
    # pre-tiled contiguous blocks per (n_chunk, ag_chunk, tile)
    a4 = adjT.reshape(NCORES, NAG, AGT, P, R)
    adjT01 = np.ascontiguousarray(
        np.stack([a4[:, :, :, :, 0:512], a4[:, :, :, :, 512:1024]], axis=1)
    )  # [cores, 2, NAG, AGT, P, 512]
    adjT2 = np.ascontiguousarray(a4[:, :, :, :, 1024:1280])  # [cores, NAG, AGT, P, 256]

    x_pad = np.zeros((NPAD, x.shape[1]), np.float32)
    x_pad[:N] = x

    in_maps = []
    for k in range(NCORES):
        xT_k = np.ascontiguousarray(x_pad[k * R : (k + 1) * R].T).astype(bfloat16)
        in_maps.append(
            {
                "xT": xT_k,
                "adjT01": adjT01[k],
                "adjT2": adjT2[k],
                "W0": np.asarray(W0, np.float32).astype(bfloat16),
                "W1": np.asarray(W1, np.float32).astype(bfloat16),
                "W2": np.asarray(W2, np.float32).astype(bfloat16),
                "b0": np.asarray(b0, np.float32).reshape(-1, 1),
                "b1": np.asarray(b1, np.float32).reshape(-1, 1),
                "b2": np.asarray(b2, np.float32).reshape(-1, 1),
            }
        )
    return in_maps


def kernel(x, edge_row, edge_col, edge_val, W0, W1, W2, b0, b1, b2):
    global LAST_RESULT
    from concourse.bass_utils import run_bass_kernel_spmd

    nc = _get_nc()
    in_maps = _preprocess(
        x, edge_row, edge_col, edge_val, W0, W1, W2, b0, b1, b2
    )
    res = run_bass_kernel_spmd(
        nc,
        in_maps,
        core_ids=list(range(NCORES)),
        trace=bool(int(os.environ.get("GCN_TRACE", "0"))),
    )
    LAST_RESULT = res

    outT = np.concatenate(
        [np.asarray(res.results[k]["outT"]) for k in range(NCORES)], axis=1
    )  # [256, 10240]
    return np.ascontiguousarray(outT.T[:N]).astype(np.float32)


# revision 8
# speedup vs baseline: 1.2430x; 1.1093x over previous
"""GCN (3-layer, skip-concat) on 8 Trainium2 NeuronCores.

Strategy (hardcoded for N=10000, E=320000, dims 512/512/256):
  - Row-partition nodes across 8 cores (1280 padded rows each, N padded
    to 10240).
  - The adjacency shard is densified on the host into A_k^T [10240, 1280]
    (bf16): SpMM becomes a dense matmul on TensorE, which beats any
    gather/scatter scheme here (random graph, 0.32% density -> every
    128-wide tile is populated).
  - Activations live feature-major (X^T) in SBUF. Per layer:
      S_k   = X^T.T @ W          (node-major [1280, d_out], PE)
      S     = AllGather(S_k)     (bf16, HBM collective, 5 chunks)
      Y^T   = S_tiles^T @ A_k^T  (feature-major, PE; S tiles stationary)
      X' ^T = [relu(Y^T + b); (Y^T + b)]   (partition-axis concat, free)
  - Software pipelining via emission order: layer L+1's dense m-tiles and
    all-gather chunks are emitted between layer L's SpMM n-chunks (each
    m-tile group only needs the columns the preceding n-chunk epilogue
    wrote), so the collectives for layer L+1 run on the CC core while
    layer L's SpMM occupies the PE.
  - Multi-k matmuls (3D APs, 4 contraction subtiles per instruction) and
    contiguous 512KB adjacency loads keep instruction counts low.
  - All matmuls bf16 with fp32 PSUM accumulation (rel err ~2e-3).
"""

import os
import numpy as np
from ml_dtypes import bfloat16

N = 10000
NPAD = 10240
NCORES = 8
R = NPAD // NCORES  # 1280 rows per core
P = 128
CT = NPAD // P  # 80 contraction tiles for the SpMM
KSUB = 4  # contraction subtiles per matmul instruction

NAG = 5  # all-gather chunks per layer
AGR = R // NAG  # 256 rows per rank per chunk
AGT = NPAD // NAG // P  # 16 contraction tiles per gathered chunk
AGG = AGT // KSUB  # 4 ksub-groups per gathered chunk

D0_IN, D0_OUT = 512, 512
D1_IN, D1_OUT = 1024, 512
D2_IN, D2_OUT = 1024, 256

# n-chunks of the 1280-wide free dim (PSUM bank = 512 fp32)
N_CHUNKS = [(0, 512), (512, 512), (1024, 256)]
# dense m-tiles whose lhsT columns come from n-chunk i's epilogue
M_OF_NC = [(0, 1, 2, 3), (4, 5, 6, 7), (8, 9)]
# all-gather chunk j consumes dense m-tiles 2j, 2j+1
AG_OF_NC = [(0, 1), (2, 3), (4,)]

_CACHE = {}
LAST_RESULT = None  # BassKernelResults of the most recent run (for test.py)


def _build_bass():
    import concourse.bass as bass
    import concourse.bacc as bacc
    import concourse.mybir as mybir
    import concourse.tile as tile

    dt = mybir.dt
    bf16 = dt.bfloat16
    f32 = dt.float32
    ts = bass.ts

    nc = bacc.Bacc(
        "TRN2",
        target_bir_lowering=False,
        debug=False,
        enable_asserts=False,
        num_devices=NCORES,
    )

    xT_d = nc.dram_tensor("xT", [D0_IN, R], bf16, kind="ExternalInput")
    # pre-tiled adjacency, [P, KSUB, nw] blocks: chunks 0,1 (512) / chunk 2 (256)
    adjT01_d = nc.dram_tensor(
        "adjT01", [2, NAG, AGG, P, KSUB, 512], bf16, kind="ExternalInput"
    )
    adjT2_d = nc.dram_tensor(
        "adjT2", [NAG, AGG, P, KSUB, 256], bf16, kind="ExternalInput"
    )
    W_d = [
        nc.dram_tensor("W0", [D0_IN, D0_OUT], bf16, kind="ExternalInput"),
        nc.dram_tensor("W1", [D1_IN, D1_OUT], bf16, kind="ExternalInput"),
        nc.dram_tensor("W2", [D2_IN, D2_OUT], bf16, kind="ExternalInput"),
    ]
    b_d = [
        nc.dram_tensor("b0", [D0_OUT, 1], f32, kind="ExternalInput"),
        nc.dram_tensor("b1", [D1_OUT, 1], f32, kind="ExternalInput"),
        nc.dram_tensor("b2", [D2_OUT, 1], f32, kind="ExternalInput"),
    ]
    outT_d = nc.dram_tensor("outT", [D2_OUT, R], f32, kind="ExternalOutput")

    DIMS = [(D0_IN, D0_OUT), (D1_IN, D1_OUT), (D2_IN, D2_OUT)]

    with tile.TileContext(nc) as tc:
        ctx_pools = (
            tc.tile_pool(name="persist", bufs=1),
            tc.tile_pool(name="work", bufs=3),
            tc.tile_pool(name="psum", bufs=1, space="PSUM"),
            tc.tile_pool(name="dram", bufs=1, space="DRAM"),
        )
        with ctx_pools[0] as persist, ctx_pools[1] as work, \
             ctx_pools[2] as psum_pool, ctx_pools[3] as dram_pool:

            # ---- resident weights / biases ----
            w_sb = []
            for L, (d_in, d_out) in enumerate(DIMS):
                wt = persist.tile(
                    [P, d_in // P, d_out], bf16, name=f"w{L}", tag=f"w{L}"
                )
                for c in range(d_in // P):
                    nc.sync.dma_start(wt[:, c, :], W_d[L][ts(c, P), :])
                w_sb.append(wt)

            b_sb = []
            for L, (d_in, d_out) in enumerate(DIMS):
                tiles = []
                for pi in range(d_out // P):
                    bt = persist.tile([P, 1], f32, name=f"b{L}_{pi}", tag=f"b{L}_{pi}")
                    nc.sync.dma_start(bt[:], b_d[L][ts(pi, P), :])
                    tiles.append(bt)
                b_sb.append(tiles)

            # ---- activations X^T (feature-major), one 3D tile per layer ----
            xt0 = persist.tile([P, D0_IN // P, R], bf16, name="xt0", tag="xt0")
            for c in range(D0_IN // P):
                nc.sync.dma_start(xt0[:, c, :], xT_d[ts(c, P), :])
            xt1 = persist.tile([P, D1_IN // P, R], bf16, name="xt1", tag="xt1")
            xt2 = persist.tile([P, D2_IN // P, R], bf16, name="xt2", tag="xt2")
            xts = [xt0, xt1, xt2]

            # gathered S, one tile per all-gather chunk (chunk-granular WAR)
            s_ch = [
                persist.tile([P, AGT, 512], bf16, name=f"s_ch{j}", tag=f"s_ch{j}")
                for j in range(NAG)
            ]

            s_bounce = [
                dram_pool.tile([R, d_out], bf16, name=f"s_bounce{L}", tag=f"sb{L}")
                for L, (_, d_out) in enumerate(DIMS)
            ]
            s_all = [
                [
                    dram_pool.tile(
                        [NCORES * AGR, d_out],
                        bf16,
                        name=f"s_all{L}_{j}",
                        tag=f"sa{L}_{j}",
                        addr_space="Shared",
                    )
                    for j in range(NAG)
                ]
                for L, (_, d_out) in enumerate(DIMS)
            ]

            def dense_m(L, m):
                """dense S_k m-tile: psum = xt.T @ W, cast to bf16, to bounce."""
                d_in, d_out = DIMS[L]
                n_ct = d_in // P
                dps = psum_pool.tile(
                    [P, d_out], f32, name=f"dps_{L}_{m}", tag="dense_ps", bufs=2
                )
                for c in range(n_ct):
                    nc.tensor.matmul(
                        dps[:],
                        lhsT=xts[L][:, c, ts(m, P)],
                        rhs=w_sb[L][:, c, :],
                        start=(c == 0),
                        stop=(c == n_ct - 1),
                    )
                s_sb = work.tile(
                    [P, d_out], bf16, name=f"ssb_{L}_{m}", tag="s_sb", bufs=4
                )
                nc.vector.tensor_copy(s_sb[:], dps[:])
                nc.sync.dma_start(s_bounce[L][ts(m, P), :], s_sb[:])

            def ag_issue(L, j):
                """all-gather chunk j of layer L's S (writes s_all only)."""
                nc.gpsimd.collective_compute(
                    "AllGather",
                    mybir.AluOpType.bypass,
                    replica_groups=[list(range(NCORES))],
                    ins=[s_bounce[L][ts(j, AGR), :].opt()],
                    outs=[s_all[L][j].opt()],
                )

            def s_load(L, j):
                """load gathered chunk j into SBUF. Must be emitted after all
                program-order readers of s_ch[j] from the previous layer."""
                d_out = DIMS[L][1]
                src = s_all[L][j].rearrange("(t p) d -> p t d", p=P)
                nc.sync.dma_start(s_ch[j][:, :, :d_out], src)

            def spmm_nc(L, nci, sink):
                """SpMM n-chunk nci of layer L + epilogue via sink()."""
                d_out = DIMS[L][1]
                n_po = d_out // P
                n0, nw = N_CHUNKS[nci]
                sp_ps = [
                    psum_pool.tile(
                        [P, nw], f32, name=f"sp_{L}_{nci}_{p}", tag=f"sp{p}"
                    )
                    for p in range(n_po)
                ]
                for j in range(NAG):
                    for g in range(AGG):
                        if nci < 2:
                            at = work.tile(
                                [P, KSUB, 512], bf16,
                                name=f"at_{L}_{nci}_{j}_{g}", tag="at", bufs=5,
                            )
                            nc.sync.dma_start(at[:], adjT01_d[nci, j, g])
                        else:
                            at = work.tile(
                                [P, KSUB, 256], bf16,
                                name=f"at2_{L}_{j}_{g}", tag="at2", bufs=4,
                            )
                            nc.sync.dma_start(at[:], adjT2_d[j, g])
                        for s in range(KSUB):
                            for p in range(n_po):
                                nc.tensor.matmul(
                                    sp_ps[p][:],
                                    lhsT=s_ch[j][:, g * KSUB + s, ts(p, P)],
                                    rhs=at[:, s, :],
                                    start=(j == 0 and g == 0 and s == 0),
                                    stop=(
                                        j == NAG - 1
                                        and g == AGG - 1
                                        and s == KSUB - 1
                                    ),
                                )
                for p in range(n_po):
                    sink(p, sp_ps[p], n0, nw)

            def sink_mid(L):
                n_po = DIMS[L][1] // P

                def sink(p, ps, n0, nw):
                    nc.scalar.activation(
                        xts[L + 1][:, p, n0 : n0 + nw],
                        ps[:],
                        mybir.ActivationFunctionType.Relu,
                        bias=b_sb[L][p][:],
                    )
                    nc.vector.tensor_scalar_add(
                        xts[L + 1][:, n_po + p, n0 : n0 + nw],
                        ps[:],
                        b_sb[L][p][:],
                    )

                return sink

            def sink_out(p, ps, n0, nw):
                ot = work.tile([P, nw], f32, name=f"ot_{n0}_{p}", tag="ot", bufs=3)
                nc.vector.tensor_scalar_add(ot[:], ps[:], b_sb[2][p][:])
                nc.sync.dma_start(outT_d[ts(p, P), n0 : n0 + nw], ot[:])

            # ================= pipeline =================
            # layer 0 dense + gathers (first use of s_ch: load immediately)
            for j in range(NAG):
                dense_m(0, 2 * j)
                dense_m(0, 2 * j + 1)
                ag_issue(0, j)
                s_load(0, j)
            # layer L spmm interleaved with layer L+1 dense + gather issue;
            # the s_ch loads must come after L's last spmm reads (program
            # order = Tile trace order), so they sit before L+1's spmm.
            for L in (0, 1):
                for nci in range(3):
                    spmm_nc(L, nci, sink_mid(L))
                    for m in M_OF_NC[nci]:
                        dense_m(L + 1, m)
                    for j in AG_OF_NC[nci]:
                        ag_issue(L + 1, j)
                for j in range(NAG):
                    s_load(L + 1, j)
            for nci in range(3):
                spmm_nc(2, nci, sink_out)

    nc.compile()
    return nc


def _get_nc():
    if "nc" not in _CACHE:
        _CACHE["nc"] = _build_bass()
    return _CACHE["nc"]


def _preprocess(x, edge_row, edge_col, edge_val, W0, W1, W2, b0, b1, b2):
    x = np.asarray(x, np.float32)
    edge_row = np.asarray(edge_row, np.int64)
    edge_col = np.asarray(edge_col, np.int64)
    edge_val = np.asarray(edge_val, np.float32)

    # contraction permutation for the chunked all-gather:
    # new index j*2048 + k*256 + r  <->  old (global node) k*1280 + j*256 + r
    jj, kk, rr = np.meshgrid(
        np.arange(NAG), np.arange(NCORES), np.arange(AGR), indexing="ij"
    )
    new_of_old = np.empty(NPAD, np.int64)
    new_of_old[(kk * R + jj * AGR + rr).ravel()] = (
        jj * (NCORES * AGR) + kk * AGR + rr
    ).ravel()

    # dense per-core adjacency blocks, transposed + permuted:
    # adjT[k][new_of_old[c], r_local] = sum of vals of edges (k*R+r_local, c)
    adjT = np.zeros((NCORES, NPAD, R), np.float32)
    core = edge_row // R
    r_local = edge_row % R
    np.add.at(adjT, (core, new_of_old[edge_col], r_local), edge_val)
    adjT = adjT.astype(bfloat16)
    # pre-tiled [P, KSUB, nw] contiguous blocks per (n_chunk, ag_chunk, group)
    a6 = adjT.reshape(NCORES, NAG, AGG, KSUB, P, R).transpose(0, 1, 2, 4, 3, 5)
    # a6: [cores, NAG, AGG, P, KSUB, R]
    adjT01 = np.ascontiguousarray(
        np.stack([a6[..., 0:512], a6[..., 512:1024]], axis=1)
    )  # [cores, 2, NAG, AGG, P, KSUB, 512]
    adjT2 = np.ascontiguousarray(a6[..., 1024:1280])  # [cores, NAG, AGG, P, KSUB, 256]

    x_pad = np.zeros((NPAD, x.shape[1]), np.float32)
    x_pad[:N] = x

    in_maps = []
    for k in range(NCORES):
        xT_k = np.ascontiguousarray(x_pad[k * R : (k + 1) * R].T).astype(bfloat16)
        in_maps.append(
            {
                "xT": xT_k,
                "adjT01": adjT01[k],
                "adjT2": adjT2[k],
                "W0": np.asarray(W0, np.float32).astype(bfloat16),
                "W1": np.asarray(W1, np.float32).astype(bfloat16),
                "W2": np.asarray(W2, np.float32).astype(bfloat16),
                "b0": np.asarray(b0, np.float32).reshape(-1, 1),
                "b1": np.asarray(b1, np.float32).reshape(-1, 1),
                "b2": np.asarray(b2, np.float32).reshape(-1, 1),
            }
        )
    return in_maps


def kernel(x, edge_row, edge_col, edge_val, W0, W1, W2, b0, b1, b2):
    global LAST_RESULT
    from concourse.bass_utils import run_bass_kernel_spmd

    nc = _get_nc()
    in_maps = _preprocess(
        x, edge_row, edge_col, edge_val, W0, W1, W2, b0, b1, b2
    )
    res = run_bass_kernel_spmd(
        nc,
        in_maps,
        core_ids=list(range(NCORES)),
        trace=bool(int(os.environ.get("GCN_TRACE", "0"))),
    )
    LAST_RESULT = res

    outT = np.concatenate(
        [np.asarray(res.results[k]["outT"]) for k in range(NCORES)], axis=1
    )  # [256, 10240]
    return np.ascontiguousarray(outT.T[:N]).astype(np.float32)


# revision 9
# speedup vs baseline: 1.3530x; 1.0885x over previous
"""GCN (3-layer, skip-concat) on 8 Trainium2 NeuronCores.

Strategy (hardcoded for N=10000, E=320000, dims 512/512/256):
  - Row-partition nodes across 8 cores (1280 padded rows each, N padded
    to 10240).
  - The adjacency shard is densified on the host into A_k^T [10240, 1280]
    (bf16): SpMM becomes a dense matmul on TensorE, which beats any
    gather/scatter scheme here (random graph, 0.32% density -> every
    128-wide tile is populated).
  - Activations live feature-major (X^T) in SBUF. Per layer:
      S_k   = X^T.T @ W          (node-major [1280, d_out], PE)
      S     = AllGather(S_k)     (bf16, HBM collective, 5 chunks)
      Y^T   = S_tiles^T @ A_k^T  (feature-major, PE; S tiles stationary)
      X' ^T = [relu(Y^T + b); (Y^T + b)]   (partition-axis concat, free)
  - Software pipelining via emission order: layer L+1's dense m-tiles and
    all-gather chunks are emitted between layer L's SpMM n-chunks (each
    m-tile group only needs the columns the preceding n-chunk epilogue
    wrote), so the collectives for layer L+1 run on the CC core while
    layer L's SpMM occupies the PE.
  - Multi-k matmuls (3D APs, 4 contraction subtiles per instruction) and
    contiguous 512KB adjacency loads keep instruction counts low.
  - All matmuls bf16 with fp32 PSUM accumulation (rel err ~2e-3).
"""

import os
import numpy as np
from ml_dtypes import bfloat16

N = 10000
NPAD = 10240
NCORES = 8
R = NPAD // NCORES  # 1280 rows per core
P = 128
CT = NPAD // P  # 80 contraction tiles for the SpMM
KSUB = 4  # contraction subtiles per matmul instruction

NAG = 5  # all-gather chunks per layer
AGR = R // NAG  # 256 rows per rank per chunk
AGT = NPAD // NAG // P  # 16 contraction tiles per gathered chunk
AGG = AGT // KSUB  # 4 ksub-groups per gathered chunk

D0_IN, D0_OUT = 512, 512
D1_IN, D1_OUT = 1024, 512
D2_IN, D2_OUT = 1024, 256

# n-chunks of the 1280-wide free dim (PSUM bank = 512 fp32)
N_CHUNKS = [(0, 512), (512, 512), (1024, 256)]
# dense m-tiles whose lhsT columns come from n-chunk i's epilogue
M_OF_NC = [(0, 1, 2, 3), (4, 5, 6, 7), (8, 9)]
# all-gather chunk j consumes dense m-tiles 2j, 2j+1
AG_OF_NC = [(0, 1), (2, 3), (4,)]

_CACHE = {}
LAST_RESULT = None  # BassKernelResults of the most recent run (for test.py)


def _build_bass():
    import concourse.bass as bass
    import concourse.bacc as bacc
    import concourse.mybir as mybir
    import concourse.tile as tile

    dt = mybir.dt
    bf16 = dt.bfloat16
    f32 = dt.float32
    ts = bass.ts

    nc = bacc.Bacc(
        "TRN2",
        target_bir_lowering=False,
        debug=False,
        enable_asserts=False,
        num_devices=NCORES,
    )

    xTf_d = nc.dram_tensor("xTf", [CT, P, D0_IN // P, P], bf16, kind="ExternalInput")
    # pre-tiled adjacency, [P, KSUB, nw] blocks: chunks 0,1 (512) / chunk 2 (256)
    adjT01_d = nc.dram_tensor(
        "adjT01", [2, NAG, AGG, P, KSUB, 512], bf16, kind="ExternalInput"
    )
    adjT2_d = nc.dram_tensor(
        "adjT2", [NAG, AGG, P, KSUB, 256], bf16, kind="ExternalInput"
    )
    W_d = [
        nc.dram_tensor("W0", [D0_IN, D0_OUT], bf16, kind="ExternalInput"),
        nc.dram_tensor("W1", [D1_IN, D1_OUT], bf16, kind="ExternalInput"),
        nc.dram_tensor("W2", [D2_IN, D2_OUT], bf16, kind="ExternalInput"),
    ]
    b_d = [
        nc.dram_tensor("b0", [D0_OUT, 1], f32, kind="ExternalInput"),
        nc.dram_tensor("b1", [D1_OUT, 1], f32, kind="ExternalInput"),
        nc.dram_tensor("b2", [D2_OUT, 1], f32, kind="ExternalInput"),
    ]
    outT_d = nc.dram_tensor("outT", [D2_OUT, R], f32, kind="ExternalOutput")

    DIMS = [(D0_IN, D0_OUT), (D1_IN, D1_OUT), (D2_IN, D2_OUT)]

    with tile.TileContext(nc) as tc:
        ctx_pools = (
            tc.tile_pool(name="persist", bufs=1),
            tc.tile_pool(name="work", bufs=3),
            tc.tile_pool(name="psum", bufs=1, space="PSUM"),
            tc.tile_pool(name="dram", bufs=1, space="DRAM"),
        )
        with ctx_pools[0] as persist, ctx_pools[1] as work, \
             ctx_pools[2] as psum_pool, ctx_pools[3] as dram_pool:

            # ---- resident weights / biases ----
            w_sb = []
            for L, (d_in, d_out) in enumerate(DIMS):
                wt = persist.tile(
                    [P, d_in // P, d_out], bf16, name=f"w{L}", tag=f"w{L}"
                )
                for c in range(d_in // P):
                    nc.sync.dma_start(wt[:, c, :], W_d[L][ts(c, P), :])
                w_sb.append(wt)

            b_sb = []
            for L, (d_in, d_out) in enumerate(DIMS):
                tiles = []
                for pi in range(d_out // P):
                    bt = persist.tile([P, 1], f32, name=f"b{L}_{pi}", tag=f"b{L}_{pi}")
                    nc.sync.dma_start(bt[:], b_d[L][ts(pi, P), :])
                    tiles.append(bt)
                b_sb.append(tiles)

            # ---- activations X^T (feature-major), one 3D tile per layer ----
            xt1 = persist.tile([P, D1_IN // P, R], bf16, name="xt1", tag="xt1")
            xt2 = persist.tile([P, D2_IN // P, R], bf16, name="xt2", tag="xt2")
            xts = [None, xt1, xt2]

            # gathered S, one tile per all-gather chunk (chunk-granular WAR)
            s_ch = [
                persist.tile([P, AGT, 512], bf16, name=f"s_ch{j}", tag=f"s_ch{j}")
                for j in range(NAG)
            ]

            s_bounce = [
                dram_pool.tile([R, d_out], bf16, name=f"s_bounce{L}", tag=f"sb{L}")
                for L, (_, d_out) in enumerate(DIMS)
            ]
            s_all = [
                [
                    dram_pool.tile(
                        [NCORES * AGR, d_out],
                        bf16,
                        name=f"s_all{L}_{j}",
                        tag=f"sa{L}_{j}",
                        addr_space="Shared",
                    )
                    for j in range(NAG)
                ]
                for L, (_, d_out) in enumerate(DIMS)
            ]

            def dense_m(L, m):
                """dense S_k m-tile: psum = xt.T @ W, cast to bf16, to bounce."""
                d_in, d_out = DIMS[L]
                n_ct = d_in // P
                dps = psum_pool.tile(
                    [P, d_out], f32, name=f"dps_{L}_{m}", tag="dense_ps", bufs=2
                )
                for c in range(n_ct):
                    nc.tensor.matmul(
                        dps[:],
                        lhsT=xts[L][:, c, ts(m, P)],
                        rhs=w_sb[L][:, c, :],
                        start=(c == 0),
                        stop=(c == n_ct - 1),
                    )
                s_sb = work.tile(
                    [P, d_out], bf16, name=f"ssb_{L}_{m}", tag="s_sb", bufs=4
                )
                nc.vector.tensor_copy(s_sb[:], dps[:])
                nc.sync.dma_start(s_bounce[L][ts(m, P), :], s_sb[:])

            def ag_issue(L, j):
                """all-gather chunk j of layer L's S (writes s_all only)."""
                nc.gpsimd.collective_compute(
                    "AllGather",
                    mybir.AluOpType.bypass,
                    replica_groups=[list(range(NCORES))],
                    ins=[s_bounce[L][ts(j, AGR), :].opt()],
                    outs=[s_all[L][j].opt()],
                )

            def s_load(L, j):
                """load gathered chunk j into SBUF. Must be emitted after all
                program-order readers of s_ch[j] from the previous layer."""
                d_out = DIMS[L][1]
                src = s_all[L][j].rearrange("(t p) d -> p t d", p=P)
                nc.sync.dma_start(s_ch[j][:, :, :d_out], src)

            def spmm_nc(L, nci, sink):
                """SpMM n-chunk nci of layer L + epilogue via sink()."""
                d_out = DIMS[L][1]
                n_po = d_out // P
                n0, nw = N_CHUNKS[nci]
                sp_ps = [
                    psum_pool.tile(
                        [P, nw], f32, name=f"sp_{L}_{nci}_{p}", tag=f"sp{p}"
                    )
                    for p in range(n_po)
                ]
                for j in range(NAG):
                    for g in range(AGG):
                        if nci < 2:
                            at = work.tile(
                                [P, KSUB, 512], bf16,
                                name=f"at_{L}_{nci}_{j}_{g}", tag="at", bufs=5,
                            )
                            nc.sync.dma_start(at[:], adjT01_d[nci, j, g])
                        else:
                            at = work.tile(
                                [P, KSUB, 256], bf16,
                                name=f"at2_{L}_{j}_{g}", tag="at2", bufs=4,
                            )
                            nc.sync.dma_start(at[:], adjT2_d[j, g])
                        for s in range(KSUB):
                            for p in range(n_po):
                                nc.tensor.matmul(
                                    sp_ps[p][:],
                                    lhsT=s_ch[j][:, g * KSUB + s, ts(p, P)],
                                    rhs=at[:, s, :],
                                    start=(j == 0 and g == 0 and s == 0),
                                    stop=(
                                        j == NAG - 1
                                        and g == AGG - 1
                                        and s == KSUB - 1
                                    ),
                                )
                for p in range(n_po):
                    sink(p, sp_ps[p], n0, nw)

            def sink_mid(L):
                n_po = DIMS[L][1] // P

                def sink(p, ps, n0, nw):
                    nc.scalar.activation(
                        xts[L + 1][:, p, n0 : n0 + nw],
                        ps[:],
                        mybir.ActivationFunctionType.Relu,
                        bias=b_sb[L][p][:],
                    )
                    nc.vector.tensor_scalar_add(
                        xts[L + 1][:, n_po + p, n0 : n0 + nw],
                        ps[:],
                        b_sb[L][p][:],
                    )

                return sink

            def sink_out(p, ps, n0, nw):
                ot = work.tile([P, nw], f32, name=f"ot_{n0}_{p}", tag="ot", bufs=3)
                nc.vector.tensor_scalar_add(ot[:], ps[:], b_sb[2][p][:])
                nc.sync.dma_start(outT_d[ts(p, P), n0 : n0 + nw], ot[:])

            # ================= pipeline =================
            # layer 0: every core computes the FULL S0 = x @ W0 locally
            # (redundant across cores) straight into s_ch -- no collective,
            # so the first (skew-absorbing) all-gather is layer 1's, which
            # has pipeline slack.
            for mt in range(CT):
                xtile = work.tile(
                    [P, D0_IN // P, P], bf16, name=f"xtile_{mt}", tag="xtile",
                    bufs=6,
                )
                nc.sync.dma_start(xtile[:], xTf_d[mt])
                dps = psum_pool.tile(
                    [P, D0_OUT], f32, name=f"dps0_{mt}", tag="dense_ps", bufs=2
                )
                for c in range(D0_IN // P):
                    nc.tensor.matmul(
                        dps[:],
                        lhsT=xtile[:, c, :],
                        rhs=w_sb[0][:, c, :],
                        start=(c == 0),
                        stop=(c == D0_IN // P - 1),
                    )
                nc.vector.tensor_copy(s_ch[mt // AGT][:, mt % AGT, :], dps[:])
            # layer L spmm interleaved with layer L+1 dense + gather issue;
            # the s_ch loads must come after L's last spmm reads (program
            # order = Tile trace order), so they sit before L+1's spmm.
            for L in (0, 1):
                for nci in range(3):
                    spmm_nc(L, nci, sink_mid(L))
                    for m in M_OF_NC[nci]:
                        dense_m(L + 1, m)
                    for j in AG_OF_NC[nci]:
                        ag_issue(L + 1, j)
                for j in range(NAG):
                    s_load(L + 1, j)
            for nci in range(3):
                spmm_nc(2, nci, sink_out)

    nc.compile()
    return nc


def _get_nc():
    if "nc" not in _CACHE:
        _CACHE["nc"] = _build_bass()
    return _CACHE["nc"]


def _preprocess(x, edge_row, edge_col, edge_val, W0, W1, W2, b0, b1, b2):
    x = np.asarray(x, np.float32)
    edge_row = np.asarray(edge_row, np.int64)
    edge_col = np.asarray(edge_col, np.int64)
    edge_val = np.asarray(edge_val, np.float32)

    # contraction permutation for the chunked all-gather:
    # new index j*2048 + k*256 + r  <->  old (global node) k*1280 + j*256 + r
    jj, kk, rr = np.meshgrid(
        np.arange(NAG), np.arange(NCORES), np.arange(AGR), indexing="ij"
    )
    new_of_old = np.empty(NPAD, np.int64)
    new_of_old[(kk * R + jj * AGR + rr).ravel()] = (
        jj * (NCORES * AGR) + kk * AGR + rr
    ).ravel()

    # dense per-core adjacency blocks, transposed + permuted:
    # adjT[k][new_of_old[c], r_local] = sum of vals of edges (k*R+r_local, c)
    adjT = np.zeros((NCORES, NPAD, R), np.float32)
    core = edge_row // R
    r_local = edge_row % R
    np.add.at(adjT, (core, new_of_old[edge_col], r_local), edge_val)
    adjT = adjT.astype(bfloat16)
    # pre-tiled [P, KSUB, nw] contiguous blocks per (n_chunk, ag_chunk, group)
    a6 = adjT.reshape(NCORES, NAG, AGG, KSUB, P, R).transpose(0, 1, 2, 4, 3, 5)
    # a6: [cores, NAG, AGG, P, KSUB, R]
    adjT01 = np.ascontiguousarray(
        np.stack([a6[..., 0:512], a6[..., 512:1024]], axis=1)
    )  # [cores, 2, NAG, AGG, P, KSUB, 512]
    adjT2 = np.ascontiguousarray(a6[..., 1024:1280])  # [cores, NAG, AGG, P, KSUB, 256]

    x_pad = np.zeros((NPAD, x.shape[1]), np.float32)
    x_pad[:N] = x
    old_of_new = np.empty(NPAD, np.int64)
    old_of_new[new_of_old] = np.arange(NPAD)
    xp4 = x_pad[old_of_new].reshape(CT, P, x.shape[1] // P, P)  # [mt, n, c, pf]
    xTf = np.ascontiguousarray(xp4.transpose(0, 3, 2, 1)).astype(bfloat16)

    in_maps = []
    for k in range(NCORES):
        in_maps.append(
            {
                "xTf": xTf,
                "adjT01": adjT01[k],
                "adjT2": adjT2[k],
                "W0": np.asarray(W0, np.float32).astype(bfloat16),
                "W1": np.asarray(W1, np.float32).astype(bfloat16),
                "W2": np.asarray(W2, np.float32).astype(bfloat16),
                "b0": np.asarray(b0, np.float32).reshape(-1, 1),
                "b1": np.asarray(b1, np.float32).reshape(-1, 1),
                "b2": np.asarray(b2, np.float32).reshape(-1, 1),
            }
        )
    return in_maps


def kernel(x, edge_row, edge_col, edge_val, W0, W1, W2, b0, b1, b2):
    global LAST_RESULT
    from concourse.bass_utils import run_bass_kernel_spmd

    nc = _get_nc()
    in_maps = _preprocess(
        x, edge_row, edge_col, edge_val, W0, W1, W2, b0, b1, b2
    )
    res = run_bass_kernel_spmd(
        nc,
        in_maps,
        core_ids=list(range(NCORES)),
        trace=bool(int(os.environ.get("GCN_TRACE", "0"))),
    )
    LAST_RESULT = res

    outT = np.concatenate(
        [np.asarray(res.results[k]["outT"]) for k in range(NCORES)], axis=1
    )  # [256, 10240]
    return np.ascontiguousarray(outT.T[:N]).astype(np.float32)
